# revision 1
# baseline (speedup 1.0000x reference)
"""MinRNN Trainium2 kernel.

Model (per batch row b):
    z_t = tanh(x_t @ W_in^T + b_in)                      # no recurrence
    u_t = sigmoid(s_{t-1} @ W_rec^T + z_t @ U_z^T + b_u) # recurrent gate
    s_t = u_t * s_{t-1} + (1 - u_t) * z_t

Strategy: data-parallel over batch across 8 cores (2 rows/core).  Each core:
  1. GEMM1: z^T = tanh(W_in @ x^T + b_in)   (dense, h-on-partitions layout)
  2. GEMM2: a^T = U_z @ z^T + b_u           (dense)
  3. Sequential scan over T with:
       - a_t injected into PSUM via an identity-weight matmul (off critical path)
       - 16 accumulating matmuls per step: W_rec^T chunks stationary (bf16),
         state (bf16) as the 2-wide moving operand; output lands h-on-partitions
       - ACT sigmoid straight from PSUM
       - DVE blend: d = s - z (off path), m = u*d, s' = z + m (bf16 + fp32 outs)

All tensors on-chip live in [128 partitions = h (mod 128), free = (h_chunk, b)]
layout so ACT/DVE ops are cheap.  State is kept in fp32 (oblk / s32keep); only
the matmul operand copy is bf16.
"""

import numpy as np
import ml_dtypes

import concourse.bass as bass
import concourse.mybir as mybir
import concourse.tile as tile
import concourse.bacc as bacc
from concourse import bass_utils

AF = mybir.ActivationFunctionType
ET = mybir.EngineType

B, T, I, H = 16, 2048, 512, 512
N_CORES = 8
BL = B // N_CORES          # batch rows per core (2)
KC = I // 128              # input-dim chunks (4)
HC = H // 128              # hidden-dim chunks (4)
TB = 128                   # scan time-block (steps per For_i iteration)

f32 = mybir.dt.float32
f32r = mybir.dt.float32r   # fp32 storage, fast reduced-precision matmul
bf16 = mybir.dt.bfloat16


def build(t_steps: int = T, tb: int = TB, compile: bool = True):
    """Build the per-core Bass program (same program on all 8 cores)."""
    assert t_steps % tb == 0
    tw = min(512, t_steps * BL)      # GEMM token-tile width (tokens = t*BL)
    assert (t_steps * BL) % tw == 0

    nc = bacc.Bacc("TRN2", target_bir_lowering=False, debug=False)

    xT = nc.dram_tensor("xT", [KC, 128, t_steps, BL], f32r, kind="ExternalInput")
    winT = nc.dram_tensor("winT", [KC, 128, H], f32r, kind="ExternalInput")
    wrecT = nc.dram_tensor("wrecT", [HC, 128, H], bf16, kind="ExternalInput")
    
    uzT = nc.dram_tensor("uzT", [HC, 128, H], bf16, kind="ExternalInput")
    bin2 = nc.dram_tensor("bin2", [HC, 128], f32, kind="ExternalInput")
    bu2 = nc.dram_tensor("bu2", [HC, 128], f32, kind="ExternalInput")
    ident = nc.dram_tensor("ident", [128, 128], bf16, kind="ExternalInput")
    out = nc.dram_tensor("outT", [BL, HC, 128, t_steps], f32r, kind="ExternalOutput")

    with tile.TileContext(nc) as tc:
        _body(tc, nc, xT, winT, wrecT, uzT, bin2, bu2, ident, out, t_steps, tb, tw)

    if compile:
        nc.compile()
    return nc


def _body(tc, nc, xT, winT, wrecT, uzT, bin2, bu2, ident, out, t_steps, tb, tw):
    from contextlib import ExitStack

    with ExitStack() as ctx:
        cpool = ctx.enter_context(tc.tile_pool(name="consts", bufs=1))
        xpool = ctx.enter_context(tc.tile_pool(name="xin", bufs=2))
        pgpool = ctx.enter_context(tc.tile_pool(name="psum_g", bufs=4, space="PSUM"))
        zgpool = ctx.enter_context(tc.tile_pool(name="zg", bufs=2))
        z16pool = ctx.enter_context(tc.tile_pool(name="z16", bufs=2))
        blkpool = ctx.enter_context(tc.tile_pool(name="blk", bufs=2))
        stpool = ctx.enter_context(tc.tile_pool(name="state", bufs=1))
        pqpool = ctx.enter_context(tc.tile_pool(name="psum_s", bufs=2, space="PSUM"))
        smpool = ctx.enter_context(tc.tile_pool(name="small", bufs=3))
        drpool = ctx.enter_context(tc.tile_pool(name="scratch", bufs=1, space="DRAM"))

        # ---- constants in SBUF ----
        w_in = cpool.tile([128, KC * H], f32r, tag="w_in")
        w_rec = cpool.tile([128, HC * H], bf16, tag="w_rec")
        u_z = cpool.tile([128, HC * H], bf16, tag="u_z")
        for k in range(KC):
            nc.sync.dma_start(w_in[:, k * H:(k + 1) * H], winT[k])
            nc.sync.dma_start(w_rec[:, k * H:(k + 1) * H], wrecT[k])
            nc.sync.dma_start(u_z[:, k * H:(k + 1) * H], uzT[k])
        idn = cpool.tile([128, 128], bf16, tag="idn")
        nc.sync.dma_start(idn[:], ident[:])
        binS = cpool.tile([128, HC], f32, tag="binS")
        nc.sync.dma_start(binS[:], bin2.ap().rearrange("c p -> p c"))
        buS = cpool.tile([128, HC], f32, tag="buS")
        nc.sync.dma_start(buS[:], bu2.ap().rearrange("c p -> p c"))

        # ---- DRAM scratch (pool tiles so Tile tracks the GEMM->scan dep) ----
        # Layout [c, p, t, b]: (t, b) is contiguous per (c, p) so block DMAs
        # merge to <=3 dims with ~1KB runs.
        zt_d = drpool.tile([HC, 128, t_steps, BL], f32r, tag="zt_d")
        ah_d = drpool.tile([HC, 128, t_steps, BL], bf16, tag="ah_d")
        al_d = drpool.tile([HC, 128, t_steps, BL], bf16, tag="al_d")
        zt_ap = zt_d[:, :, :, :]
        ah_ap = ah_d[:, :, :, :]
        al_ap = al_d[:, :, :, :]

        xr = xT.ap().rearrange("k p t b -> p k t b")
        twt = tw // BL                    # GEMM tile width in t (tokens = t*BL)

        # carry: last t-column of the previous tile's z^T (per k-chunk)
        z_last = stpool.tile([128, KC * BL], bf16, tag="z_last")
        zzero = stpool.tile([128, KC * BL], f32, tag="zzero")
        nc.vector.memset(zzero[:], 0.0)
        nc.vector.tensor_copy(z_last[:], zzero[:])

        # ---- precompute z^T and a~^T = U_z z_t + W_rec z_{t-1} + b_u ----
        for ti, ts in enumerate(range(0, t_steps, twt)):
            xs = xpool.tile([128, KC * tw], f32r, tag="xs")
            nc.sync.dma_start(
                xs[:].rearrange("p (k f) -> p k f", k=KC),
                xr[:, :, ts:ts + twt, :],
            )
            zR = z16pool.tile([128, HC * tw], f32r, tag="zR")
            zR16 = z16pool.tile([128, HC * tw], bf16, tag="zR16")
            for cm in range(HC):
                ps = pgpool.tile([128, tw], f32, tag="ps_g")
                for k in range(KC):
                    nc.tensor.matmul(
                        ps[:],
                        w_in[:, k * H + cm * 128:k * H + cm * 128 + 128],
                        xs[:, k * tw:(k + 1) * tw],
                        start=(k == 0),
                        stop=(k == KC - 1),
                    )
                zc = zR[:, cm * tw:(cm + 1) * tw]
                nc.scalar.activation(zc, ps[:], AF.Tanh,
                                     bias=binS[:, cm:cm + 1], scale=1.0)
                nc.vector.tensor_copy(zR16[:, cm * tw:(cm + 1) * tw], zc)
                nc.sync.dma_start(zt_ap[cm, :, ts:ts + twt, :], zc)
            for cm in range(HC):
                ps = pgpool.tile([128, tw], f32, tag="ps_g")
                for k in range(HC):
                    nc.tensor.matmul(
                        ps[:],
                        u_z[:, k * H + cm * 128:k * H + cm * 128 + 128],
                        zR16[:, k * tw:(k + 1) * tw],
                        start=(k == 0),
                        stop=False,
                        skip_group_check=True,
                    )
                # + W_rec @ z shifted one step back in t (tokens shift
                # by BL); boundary col from the previous tile's carried z
                # (zeros before t=0).  Shift+boundary share each weight load.
                for k in range(HC):
                    wslice = w_rec[:, k * H + cm * 128:k * H + cm * 128 + 128]
                    nc.tensor.matmul(
                        ps[:, BL:tw], wslice,
                        zR16[:, k * tw:(k + 1) * tw - BL],
                        start=False, stop=(k == HC - 1),
                        skip_group_check=True,
                    )
                    nc.tensor.matmul(
                        ps[:, 0:BL], wslice,
                        z_last[:, k * BL:(k + 1) * BL],
                        start=False, stop=(k == HC - 1),
                        skip_group_check=True,
                    )
                a32 = zgpool.tile([128, tw], f32, tag="a32")
                nc.scalar.activation(a32[:], ps[:], AF.Identity,
                                     bias=buS[:, cm:cm + 1], scale=1.0)
                ah16 = zgpool.tile([128, tw], bf16, tag="ah16")
                nc.vector.tensor_copy(ah16[:], a32[:])
                al16 = zgpool.tile([128, tw], bf16, tag="al16")
                nc.vector.tensor_sub(al16[:], a32[:], ah16[:])
                nc.sync.dma_start(ah_ap[cm, :, ts:ts + twt, :], ah16[:])
                nc.sync.dma_start(al_ap[cm, :, ts:ts + twt, :], al16[:])
            # update carry AFTER this tile's boundary matmuls consumed it
            nc.vector.tensor_copy(
                z_last[:].rearrange("p (k b) -> p k b", k=KC),
                zR16[:].rearrange("p (k t) -> p k t", k=KC)[:, :, tw - BL:tw],
            )

        # ---- sequential scan ----
        # u_pre(t) = a~_t + W_rec m_{t-1}, with m_t = u_t (s_{t-1} - z_t)
        # and s_t = z_t + m_t.  m (small magnitude) is the only bf16-rounded
        # recurrent quantity; a~ is carried as a bf16 hi+lo pair.
        m16s = [stpool.tile([128, HC * BL], bf16, tag=f"m16{i}",
                            name=f"m16{i}")
                for i in range(2)]
        s32k = stpool.tile([128, HC * BL], f32, tag="s32k")
        nc.vector.memset(m16s[0][:], 0.0)
        nc.vector.memset(s32k[:], 0.0)
        s32kv = s32k[:].rearrange("p (c b) -> p c b", b=BL)

        zr = zt_ap.rearrange("c p t b -> p c t b")
        ahr = ah_ap.rearrange("c p t b -> p c t b")
        alr = al_ap.rearrange("c p t b -> p c t b")
        outr = out.ap().rearrange("b c p t -> p b c t")

        # double-buffered block streams, manually ping-ponged so the next
        # block's loads overlap the current block's compute
        def blk_bufs(i):
            zbt = stpool.tile([128, HC * BL * tb], f32r, tag=f"zb{i}",
                              name=f"zb{i}")
            aht = stpool.tile([128, HC * BL * tb], bf16, tag=f"abh{i}",
                              name=f"abh{i}")
            alt = stpool.tile([128, HC * BL * tb], bf16, tag=f"abl{i}",
                              name=f"abl{i}")
            return (zbt, aht, alt)

        bufA, bufB = blk_bufs(0), blk_bufs(1)

        def load_block(buf, t0):
            zbt, aht, alt = buf
            nc.sync.dma_start(
                zbt[:].rearrange("p (c t b) -> p c t b", c=HC, b=BL),
                zr[:, :, bass.ds(t0, tb), :])
            nc.sync.dma_start(
                aht[:].rearrange("p (c t b) -> p c t b", c=HC, b=BL),
                ahr[:, :, bass.ds(t0, tb), :])
            nc.sync.dma_start(
                alt[:].rearrange("p (c t b) -> p c t b", c=HC, b=BL),
                alr[:, :, bass.ds(t0, tb), :])

        def run_block(buf, t0):
            zbt, aht, alt = buf
            zb3 = zbt[:].rearrange("p (c t b) -> p c t b", c=HC, b=BL)
            abh3 = aht[:].rearrange("p (c t b) -> p c t b", c=HC, b=BL)
            abl3 = alt[:].rearrange("p (c t b) -> p c t b", c=HC, b=BL)
            obs = [blkpool.tile([128, tb * HC], f32r, tag=f"ob{b}",
                                name=f"ob{b}")
                   for b in range(BL)]
            ob2 = [o[:].rearrange("p (c t) -> p c t", c=HC) for o in obs]

            for tl in range(tb):
                m_in = m16s[tl % 2]
                m_out = m16s[(tl + 1) % 2]
                ps = pqpool.tile([128, HC * BL], f32, tag="ps_s")
                # seed the whole psum tile with a~_t (hi+lo) via identity MMs
                nc.tensor.matmul(ps[:], idn[:], abh3[:, :, tl, :],
                                 start=True, stop=False, skip_group_check=True)
                nc.tensor.matmul(ps[:], idn[:], abl3[:, :, tl, :],
                                 start=False, stop=False, skip_group_check=True)
                # d_b = s_{t-1} - z_t  (off the critical path)
                d = smpool.tile([128, HC * BL], f32r, tag="d")
                dv = d[:].rearrange("p (c b) -> p c b", b=BL)
                zt_v = zb3[:, :, tl, :]
                for b in range(BL):
                    sprev = (s32kv[:, :, b] if tl == 0
                             else ob2[b][:, :, tl - 1])
                    nc.vector.tensor_sub(dv[:, :, b], sprev, zt_v[:, :, b])
                for c in range(HC):
                    pc = ps[:, c * BL:(c + 1) * BL]
                    for k in range(KC):
                        nc.tensor.matmul(
                            pc,
                            w_rec[:, k * H + c * 128:k * H + c * 128 + 128],
                            m_in[:, k * BL:(k + 1) * BL],
                            start=False,
                            stop=(k == KC - 1),
                            skip_group_check=True,
                        )
                # u = sigmoid(pre) = 0.5*(1+tanh(pre/2)); fold the 0.5
                # into d so the whole program stays on one ACT table set.
                tau = pqpool.tile([128, HC * BL], f32, tag="tau")
                nc.scalar.activation(tau[:], ps[:], AF.Tanh, scale=0.5)
                dh = smpool.tile([128, HC * BL], f32r, tag="dh")
                nc.vector.tensor_scalar_mul(dh[:], d[:], 0.5)
                # critical: m_t = (tau+1) * dh (bf16, feeds next matmuls)
                nc.vector.scalar_tensor_tensor(
                    m_out[:], tau[:], 1.0, dh[:],
                    op0=mybir.AluOpType.add, op1=mybir.AluOpType.mult)
                mv = m_out[:].rearrange("p (c b) -> p c b", b=BL)
                # off-path: s_t = z_t + m_t (fp32 outputs, per batch row)
                for b in range(BL):
                    nc.vector.tensor_add(ob2[b][:, :, tl], zt_v[:, :, b],
                                         mv[:, :, b])

            for b in range(BL):
                nc.vector.tensor_copy(s32kv[:, :, b], ob2[b][:, :, tb - 1])
                nc.sync.dma_start(outr[:, b, :, bass.ds(t0, tb)], obs[b][:])

        n_pairs = t_steps // (2 * tb)
        load_block(bufA, 0)
        if n_pairs > 1:
            with tc.For_i(0, t_steps - 2 * tb, 2 * tb,
                          hint_engines=(ET.PE, ET.DVE, ET.Activation, ET.SP,
                                        ET.Pool),
                          name="scan") as t0:
                load_block(bufB, t0 + tb)
                run_block(bufA, t0)
                load_block(bufA, t0 + 2 * tb)
                run_block(bufB, t0 + tb)
        tlast = t_steps - 2 * tb
        load_block(bufB, tlast + tb)
        run_block(bufA, tlast)
        run_block(bufB, tlast + tb)


_CACHED = {}


def _get_nc(t_steps=T, tb=TB):
    key = (t_steps, tb)
    if key not in _CACHED:
        _CACHED[key] = build(t_steps, tb)
    return _CACHED[key]


def make_in_maps(inputs, W_in, b_in, W_rec, U_z, b_u, t_steps=T):
    x = np.asarray(inputs, dtype=np.float32)
    winT_np = np.ascontiguousarray(
        np.asarray(W_in, np.float32).T.reshape(KC, 128, H))
    wrecT_np = np.ascontiguousarray(
        np.asarray(W_rec, np.float32).T.reshape(HC, 128, H)).astype(ml_dtypes.bfloat16)
    uzT_np = np.ascontiguousarray(
        np.asarray(U_z, np.float32).T.reshape(HC, 128, H)).astype(ml_dtypes.bfloat16)
    bin_np = np.ascontiguousarray(np.asarray(b_in, np.float32).reshape(HC, 128))
    bu_np = np.ascontiguousarray(np.asarray(b_u, np.float32).reshape(HC, 128))
    id_np = np.eye(128, dtype=np.float32).astype(ml_dtypes.bfloat16)

    in_maps = []
    for c in range(N_CORES):
        xc = x[c * BL:(c + 1) * BL, :t_steps, :]          # (BL, t, I)
        xTc = np.ascontiguousarray(xc.transpose(2, 1, 0)  # (I, t, BL)
                                   ).reshape(KC, 128, t_steps, BL)
        in_maps.append({
            "xT": xTc, "winT": winT_np, "wrecT": wrecT_np, "uzT": uzT_np,
            "bin2": bin_np, "bu2": bu_np, "ident": id_np,
        })
    return in_maps


def kernel(inputs, W_in, b_in, W_rec, U_z, b_u):
    nc = _get_nc()
    in_maps = make_in_maps(inputs, W_in, b_in, W_rec, U_z, b_u)
    res = bass_utils.run_bass_kernel_spmd(nc, in_maps, core_ids=list(range(N_CORES)))
    outs = [unpack_out(res.results[c]["outT"]) for c in range(N_CORES)]
    return np.ascontiguousarray(np.concatenate(outs, axis=0), dtype=np.float32)


def unpack_out(oT):
    # [BL, HC, 128, t] -> [BL, t, HC*128]
    bl, hc, p, t = oT.shape
    return oT.transpose(0, 3, 1, 2).reshape(bl, t, hc * p)



# revision 4
# speedup vs baseline: 5.4947x; 5.4947x over previous
"""MinRNN Trainium2 kernel — quasi-DEER fixed-point iteration.

Model (per batch row b):
    z_t = tanh(x_t @ W_in^T + b_in)                      # no recurrence
    u_t = sigmoid(s_{t-1} @ W_rec^T + z_t @ U_z^T + b_u) # recurrent gate
    s_t = u_t * s_{t-1} + (1 - u_t) * z_t

Instead of a 2048-step sequential scan (16 tiny weight-reloading matmuls
per step => LDWEIGHTS-bound), solve the recurrence by fixed-point
iteration (quasi-DEER with a diagonal Jacobian, which for this cell is
exactly "freeze the gate trajectory, solve the linear scan exactly"):

    s^0 = z
    repeat K times:
        pre_t = W_rec s^{k-1}_{t-1} + c_t        # one WIDE GEMM over all t
        u_t   = sigmoid(pre_t)                   # one big ACT pass
        s^k   = linscan(u, (1-u) z)              # HW tensor_tensor_scan

with c = U_z z + b_u precomputed.  Each sweep amortizes the W_rec weight
loads over 512-wide moving operands, and the sequential part becomes the
DVE's native linear-scan instruction (fp32 internal state).  K=6 sweeps
converge to max-abs error ~5e-3 (validated vs fp32 reference in numpy,
including all bf16 rounding), ~4x inside the 2e-2 relative tolerance.

Data-parallel over batch: 8 cores x 2 rows.  Everything except x-in and
s-out stays in SBUF.  Layouts are [128 partitions = h (mod 128),
free = (chunk, t, b)] throughout.
"""

import numpy as np
import ml_dtypes

import concourse.bass as bass
import concourse.mybir as mybir
import concourse.tile as tile
import concourse.bacc as bacc
from concourse import bass_utils

AF = mybir.ActivationFunctionType
OP = mybir.AluOpType

B, T, I, H = 16, 2048, 512, 512
N_CORES = 8
BL = B // N_CORES          # batch rows per core (2)
KC = I // 128              # input-dim chunks (4)
HC = H // 128              # hidden-dim chunks (4)
TB = 256                   # t-steps per tile (512 moving columns)
K_SWEEPS = 6

f32 = mybir.dt.float32
f32r = mybir.dt.float32r
bf16 = mybir.dt.bfloat16


def build(t_steps: int = T, tb: int = TB, sweeps: int = K_SWEEPS,
          compile: bool = True):
    tb = min(tb, t_steps)
    assert t_steps % tb == 0

    nc = bacc.Bacc("TRN2", target_bir_lowering=False, debug=False)

    xT = nc.dram_tensor("xT", [KC, 128, t_steps, BL], f32r, kind="ExternalInput")
    winT = nc.dram_tensor("winT", [KC, 128, H], f32r, kind="ExternalInput")
    wrecT = nc.dram_tensor("wrecT", [HC, 128, H], bf16, kind="ExternalInput")
    uzT = nc.dram_tensor("uzT", [HC, 128, H], bf16, kind="ExternalInput")
    binNeg = nc.dram_tensor("binNeg", [HC, 128], f32, kind="ExternalInput")
    bu2 = nc.dram_tensor("bu2", [HC, 128], f32, kind="ExternalInput")
    ident = nc.dram_tensor("ident", [128, 128], bf16, kind="ExternalInput")
    out = nc.dram_tensor("outT", [BL, HC, 128, t_steps], f32, kind="ExternalOutput")

    with tile.TileContext(nc) as tc:
        _body(tc, nc, xT, winT, wrecT, uzT, binNeg, bu2, ident, out,
              t_steps, tb, sweeps)

    if compile:
        nc.compile()
    return nc


def _body(tc, nc, xT, winT, wrecT, uzT, binNeg, bu2, ident, out,
          t_steps, tb, sweeps):
    from contextlib import ExitStack

    nt = t_steps // tb          # number of time tiles
    tw = tb * BL                # moving columns per tile (<=512)
    F = t_steps * BL            # free columns per h-chunk

    with ExitStack() as ctx:
        cpool = ctx.enter_context(tc.tile_pool(name="consts", bufs=1))
        xpool = ctx.enter_context(tc.tile_pool(name="xin", bufs=1))
        pspool = ctx.enter_context(tc.tile_pool(name="ps", bufs=8, space="PSUM"))
        ugpool = ctx.enter_context(tc.tile_pool(name="ug", bufs=2))
        sfpool = ctx.enter_context(tc.tile_pool(name="sfin", bufs=2))

        # ---- constants ----
        w_in = cpool.tile([128, KC * H], f32r, tag="w_in")
        w_rec = cpool.tile([128, HC * H], bf16, tag="w_rec")
        u_z = cpool.tile([128, HC * H], bf16, tag="u_z")
        for k in range(KC):
            nc.sync.dma_start(w_in[:, k * H:(k + 1) * H], winT[k])
            nc.sync.dma_start(w_rec[:, k * H:(k + 1) * H], wrecT[k])
            nc.sync.dma_start(u_z[:, k * H:(k + 1) * H], uzT[k])
        idn = cpool.tile([128, 128], bf16, tag="idn")
        nc.sync.dma_start(idn[:], ident[:])
        binS = cpool.tile([128, HC], f32, tag="binS")
        nc.sync.dma_start(binS[:], binNeg.ap().rearrange("c p -> p c"))
        buS = cpool.tile([128, HC], f32, tag="buS")
        nc.sync.dma_start(buS[:], bu2.ap().rearrange("c p -> p c"))
        zzero = cpool.tile([128, 1], f32, tag="zzero")
        nc.vector.memset(zzero[:], 0.0)
        carry = cpool.tile([128, HC * BL], f32, tag="carry")

        # ---- persistent activations (SBUF-resident whole trajectories) ----
        zneg = cpool.tile([128, HC * F], bf16, tag="zneg")    # -z
        cbuf = cpool.tile([128, HC * F], bf16, tag="cbuf")    # U_z z + b_u
        # state, (T+1) slots per chunk: slot j holds s_{j-1}; slot 0 = 0
        sA = cpool.tile([128, KC * (t_steps + 1) * BL], bf16, tag="sA")
        sB = cpool.tile([128, KC * (t_steps + 1) * BL], bf16, tag="sB")
        s_bufs = [sA, sB]

        zneg2 = zneg[:].rearrange("p (c f) -> p c f", c=HC)
        cb2 = cbuf[:].rearrange("p (c f) -> p c f", c=HC)
        sv4 = [s[:].rearrange("p (k t b) -> p k t b", k=KC, b=BL)
               for s in s_bufs]
        carry3 = carry[:].rearrange("p (c b) -> p c b", b=BL)
        xr = xT.ap().rearrange("k p t b -> p k t b")
        outr = out.ap().rearrange("b c p t -> p b c t")

        for s in s_bufs:            # zero the s_{-1} slot of both buffers
            v = s[:].rearrange("p (k t b) -> p k t b", k=KC, b=BL)
            nc.vector.memset(v[:, :, 0, :], 0.0)

        # ---- phase 1: z = tanh(W_in x + b_in), c = U_z z + b_u ----
        for ti in range(nt):
            ts = ti * tb
            fs = slice(ts * BL, ts * BL + tw)
            xs = xpool.tile([128, KC * tw], f32r, tag="xs")
            nc.sync.dma_start(
                xs[:].rearrange("p (k f) -> p k f", k=KC),
                xr[:, :, ts:ts + tb, :])
            for cm in range(HC):
                ps = pspool.tile([128, tw], f32, tag="ps")
                for k in range(KC):
                    nc.tensor.matmul(
                        ps[:],
                        w_in[:, k * H + cm * 128:k * H + cm * 128 + 128],
                        xs[:, k * tw:(k + 1) * tw],
                        start=(k == 0), stop=(k == KC - 1))
                # zneg = tanh(-pre - b_in) = -z
                nc.scalar.activation(zneg2[:, cm, fs], ps[:], AF.Tanh,
                                     bias=binS[:, cm:cm + 1], scale=-1.0)
            for cm in range(HC):
                ps = pspool.tile([128, tw], f32, tag="ps")
                for k in range(HC):
                    nc.tensor.matmul(
                        ps[:],
                        u_z[:, k * H + cm * 128:k * H + cm * 128 + 128],
                        zneg2[:, k, fs],
                        start=(k == 0), stop=(k == HC - 1))
                # c = -(U_z zneg) + b_u
                nc.scalar.activation(cb2[:, cm, fs], ps[:], AF.Identity,
                                     bias=buS[:, cm:cm + 1], scale=-1.0)
            # init s^0 = z  (slots 1+ts .. 1+ts+tb of buffer A)
            for k in range(KC):
                nc.vector.tensor_scalar_mul(
                    sv4[0][:, k, 1 + ts:1 + ts + tb, :],
                    zneg2[:, k, fs].rearrange("p (t b) -> p t b", b=BL),
                    -1.0)

        # ---- phase 2: quasi-DEER sweeps ----
        for sw in range(sweeps):
            rbuf = sv4[sw % 2]
            wbuf = sv4[(sw + 1) % 2]
            last = sw == sweeps - 1
            for ti in range(nt):
                ts = ti * tb
                fs = slice(ts * BL, ts * BL + tw)
                pss = [pspool.tile([128, tw], f32, tag="ps", name=f"ps{cm}")
                       for cm in range(HC)]
                # inject c (one idn stationary load for all 4 chunks)
                for cm in range(HC):
                    nc.tensor.matmul(pss[cm][:], idn[:], cb2[:, cm, fs],
                                     start=True, stop=False,
                                     skip_group_check=True)
                # pre += W_rec @ s_{t-1}   (k-major: stationary reuse)
                for k in range(KC):
                    for cm in range(HC):
                        nc.tensor.matmul(
                            pss[cm][:],
                            w_rec[:, k * H + cm * 128:k * H + cm * 128 + 128],
                            rbuf[:, k, ts:ts + tb, :],
                            start=False, stop=(k == KC - 1),
                            skip_group_check=True)
                ut = ugpool.tile([128, HC * tw], f32, tag="u")
                gt = ugpool.tile([128, HC * tw], f32, tag="g")
                u2 = ut[:].rearrange("p (c f) -> p c f", c=HC)
                g2 = gt[:].rearrange("p (c f) -> p c f", c=HC)
                for cm in range(HC):
                    nc.scalar.activation(u2[:, cm, :], pss[cm][:], AF.Sigmoid)
                    # g = (u - 1) * (-z) = (1-u) z
                    nc.vector.scalar_tensor_tensor(
                        g2[:, cm, :], u2[:, cm, :], 1.0, zneg2[:, cm, fs],
                        op0=OP.subtract, op1=OP.mult)
                u4 = ut[:].rearrange("p (c t b) -> p c t b", c=HC, b=BL)
                g4 = gt[:].rearrange("p (c t b) -> p c t b", c=HC, b=BL)
                if not last:
                    for cm in range(HC):
                        for b in range(BL):
                            init = (zzero[:, 0:1] if ti == 0
                                    else wbuf[:, cm, ts:ts + 1, b])
                            nc.vector.tensor_tensor_scan(
                                wbuf[:, cm, 1 + ts:1 + ts + tb, b],
                                u4[:, cm, :, b], g4[:, cm, :, b],
                                init, op0=OP.mult, op1=OP.add)
                else:
                    # (c, b, t) layout so the out-DMA reads contiguous runs
                    sf = sfpool.tile([128, HC * tw], f32, tag="sf")
                    sf4 = sf[:].rearrange("p (c b t) -> p c b t", c=HC, b=BL)
                    for cm in range(HC):
                        for b in range(BL):
                            init = (zzero[:, 0:1] if ti == 0
                                    else carry3[:, cm, b:b + 1])
                            nc.vector.tensor_tensor_scan(
                                sf4[:, cm, b, :],
                                u4[:, cm, :, b], g4[:, cm, :, b],
                                init, op0=OP.mult, op1=OP.add)
                    if ti < nt - 1:
                        nc.vector.tensor_copy(carry3[:, :, :],
                                              sf4[:, :, :, tb - 1])
                    for b in range(BL):
                        for cm in range(HC):
                            nc.sync.dma_start(outr[:, b, cm, ts:ts + tb],
                                              sf4[:, cm, b, :])


_CACHED = {}


def _get_nc(t_steps=T, tb=TB):
    key = (t_steps, tb)
    if key not in _CACHED:
        _CACHED[key] = build(t_steps, tb)
    return _CACHED[key]


def make_in_maps(inputs, W_in, b_in, W_rec, U_z, b_u, t_steps=T):
    x = np.asarray(inputs, dtype=np.float32)
    winT_np = np.ascontiguousarray(
        np.asarray(W_in, np.float32).T.reshape(KC, 128, H))
    wrecT_np = np.ascontiguousarray(
        np.asarray(W_rec, np.float32).T.reshape(HC, 128, H)).astype(ml_dtypes.bfloat16)
    uzT_np = np.ascontiguousarray(
        np.asarray(U_z, np.float32).T.reshape(HC, 128, H)).astype(ml_dtypes.bfloat16)
    binNeg_np = np.ascontiguousarray(
        (-np.asarray(b_in, np.float32)).reshape(HC, 128))
    bu_np = np.ascontiguousarray(np.asarray(b_u, np.float32).reshape(HC, 128))
    id_np = np.eye(128, dtype=np.float32).astype(ml_dtypes.bfloat16)

    in_maps = []
    for c in range(N_CORES):
        xc = x[c * BL:(c + 1) * BL, :t_steps, :]          # (BL, t, I)
        xTc = np.ascontiguousarray(xc.transpose(2, 1, 0)  # (I, t, BL)
                                   ).reshape(KC, 128, t_steps, BL)
        in_maps.append({
            "xT": xTc, "winT": winT_np, "wrecT": wrecT_np, "uzT": uzT_np,
            "binNeg": binNeg_np, "bu2": bu_np, "ident": id_np,
        })
    return in_maps


def kernel(inputs, W_in, b_in, W_rec, U_z, b_u):
    nc = _get_nc()
    in_maps = make_in_maps(inputs, W_in, b_in, W_rec, U_z, b_u)
    res = bass_utils.run_bass_kernel_spmd(nc, in_maps, core_ids=list(range(N_CORES)))
    outs = [unpack_out(res.results[c]["outT"]) for c in range(N_CORES)]
    return np.ascontiguousarray(np.concatenate(outs, axis=0), dtype=np.float32)


def unpack_out(oT):
    # [BL, HC, 128, t] -> [BL, t, HC*128]
    bl, hc, p, t = oT.shape
    return oT.transpose(0, 3, 1, 2).reshape(bl, t, hc * p)
